# revision 1
# baseline (speedup 1.0000x reference)
"""GATv2 (2-layer) + mean-pool + MLP + log_softmax on 8 TRN2 NeuronCores.

Sharding: dst-sorted edges, graph-aligned node ranges per core. Per-edge
xl[src] via dma_gather (lo/hi int16 split); xr[dst] via selection-matrix
matmul; segment-softmax denominator aggregated jointly with the numerator
(division at node level). att is folded into the tables host-side with a
column permutation (positives first) so the score reduces to Relu row-sums.
Two SPMD launches: layer 0 -> host all-gather of h -> layer 1 + pooling+MLP.
"""
import sys, os

for _p in ("/opt/trn_rl_repo", "/root/.axon_site/_ro/trn_rl_repo"):
    if os.path.isdir(_p) and _p not in sys.path:
        sys.path.insert(0, _p)

import numpy as np
from contextlib import ExitStack

import concourse.bass as bass
import concourse.bacc as bacc
import concourse.mybir as mybir
import concourse.tile as tile
from concourse.bass_utils import run_bass_kernel_spmd
from concourse.masks import make_identity

F32 = mybir.dt.float32
I16 = mybir.dt.int16
I32 = mybir.dt.int32

P = 128
N = 50000
E = 1_600_000
H = 64
IN = 7
EDIM = 4
G = 256
FC = 128
NC_CLS = 2
ALPHA = 0.2
NCORES = 8
LO = 32768                  # int16 index split
NG_PAD = 50176              # global nodes padded to x128
NT_GLOB = NG_PAD // P       # 392 global tiles
NHI = NG_PAD - LO           # 17408


# ----------------------------------------------------------------------------
# host-side prep helpers
# ----------------------------------------------------------------------------

def _wrap16(idx):
    """int16 idx list (len % 128 == 0) -> [128, len/16] dma_gather layout."""
    n = idx.shape[0]
    w = idx.reshape(n // 16, 16).T.copy()
    return np.tile(w, (8, 1))


def _fold_weights(Wl, bl, Wr, br, We, att, bias, in_perm):
    """Fold att into tables; permute hidden cols (att>0 first).

    Returns dict with folded weights, npos, order, rcp (1/att in new order).
    in_perm permutes the INPUT rows of Wl/Wr (identity for layer 0).
    """
    att = np.asarray(att)
    assert np.abs(att).min() > 1e-8
    order = np.argsort(~(att > 0), kind="stable")
    npos = int((att > 0).sum())
    assert 0 < npos < H
    Wl_f = ((Wl * att[None, :])[:, order])[in_perm, :]
    Wr_f = ((Wr * att[None, :])[:, order])[in_perm, :]
    bl_f = (bl * att)[order]
    br_f = (br * att)[order]
    We_f = (We * att[None, :])[:, order]
    bias_f = bias[order]
    rcp = 1.0 / att[order]
    return dict(Wl=Wl_f.astype(np.float32), bl=bl_f.astype(np.float32),
                Wr=Wr_f.astype(np.float32), br=br_f.astype(np.float32),
                We=We_f.astype(np.float32), bias=bias_f.astype(np.float32),
                rcp=rcp.astype(np.float32), npos=npos, order=order)


def _rep(v):
    """[H] -> [128, H] replicated f32."""
    return np.tile(np.asarray(v, np.float32)[None, :], (P, 1)).copy()


def _prep_shards(edge_index, edge_attr, batch):
    """Partition nodes (graph-aligned) and edges (by dst) across cores.

    Returns per-core arrays + shared structure (identical shapes everywhere).
    """
    src_g = np.asarray(edge_index[0], np.int64)
    dst_g = np.asarray(edge_index[1], np.int64)
    batch = np.asarray(batch, np.int64)
    ea = np.asarray(edge_attr, np.float32)

    # graph-aligned node ranges ~N/8 each
    gfirst = np.searchsorted(batch, np.arange(G + 1))   # node start of graph g
    tgt = np.linspace(0, N, NCORES + 1)
    cut_g = [int(np.argmin(np.abs(gfirst - t))) for t in tgt]
    cut_g[0], cut_g[-1] = 0, G
    node_cut = gfirst[cut_g]                            # node boundaries, len 9
    n_per = np.diff(node_cut)
    T = int(np.ceil(n_per.max() / P))                   # local tiles per core
    NODES_PAD = T * P

    order_e = np.argsort(dst_g, kind="stable")
    src_s, dst_s, ea_s = src_g[order_e], dst_g[order_e], ea[order_e]
    core_e0 = np.searchsorted(dst_s, node_cut)          # edge ranges per core

    # per (core, tile): lo/hi chunk counts -> shared maxes
    c_lo = np.zeros((NCORES, T), np.int64)
    c_hi = np.zeros((NCORES, T), np.int64)
    tile_edges = [[None] * T for _ in range(NCORES)]
    for c in range(NCORES):
        s0 = node_cut[c]
        es, ee = core_e0[c], core_e0[c + 1]
        dloc_all = dst_s[es:ee] - s0
        t_of = dloc_all // P
        t0 = np.searchsorted(t_of, np.arange(T + 1))
        for j in range(T):
            a, b = es + t0[j], es + t0[j + 1]
            lo_m = src_s[a:b] < LO
            tile_edges[c][j] = (a, b, lo_m)
            c_lo[c, j] = int(np.ceil(lo_m.sum() / P))
            c_hi[c, j] = int(np.ceil((~lo_m).sum() / P))
    C_LO = c_lo.max(axis=0)
    C_HI = c_hi.max(axis=0)
    C_LO = np.maximum(C_LO, 1)
    C_HI = np.maximum(C_HI, 1)
    C_T = C_LO + C_HI
    C_TOT = int(C_T.sum())

    idx16 = np.zeros((NCORES, 128, C_TOT * 8), np.int16)
    idxd16 = np.zeros((NCORES, 128, C_TOT * 8), np.int16)
    dloc_f = np.full((NCORES, 128, C_TOT), 999.0, np.float32)
    eaT = np.zeros((NCORES, EDIM, C_TOT * P), np.float32)
    for c in range(NCORES):
        s0 = node_cut[c]
        off = 0
        for j in range(T):
            a, b, lo_m = tile_edges[c][j]
            sj, dj, ej = src_s[a:b], dst_s[a:b], ea_s[a:b]
            for half, m, cnt in ((0, lo_m, C_LO[j]), (1, ~lo_m, C_HI[j])):
                n_real = int(m.sum())
                npad = cnt * P
                sidx = np.zeros(npad, np.int64)
                sidx[:n_real] = sj[m] - (LO if half else 0)
                dl = np.full(npad, 999.0, np.float32)
                dl[:n_real] = (dj[m] - s0 - j * P).astype(np.float32)
                eab = np.zeros((npad, EDIM), np.float32)
                eab[:n_real] = ej[m]
                didx = np.zeros(npad, np.int64)
                didx[:n_real] = dj[m] - s0
                idx16[c, :, off * 8:(off + cnt) * 8] = _wrap16(sidx.astype(np.int16))
                idxd16[c, :, off * 8:(off + cnt) * 8] = _wrap16(didx.astype(np.int16))
                dloc_f[c, :, off:off + cnt] = dl.reshape(cnt, P).T
                eaT[c, :, off * P:(off + cnt) * P] = eab.T
                off += cnt

    # pooling structure
    G_T = int(np.diff(cut_g).max())
    batch_loc = np.full((NCORES, 128, T), 999.0, np.float32)
    for c in range(NCORES):
        s0, n_c = node_cut[c], n_per[c]
        bl = np.full(NODES_PAD, 999.0, np.float32)
        bl[:n_c] = (batch[s0:s0 + n_c] - cut_g[c]).astype(np.float32)
        batch_loc[c] = bl.reshape(T, P).T

    return dict(node_cut=node_cut, cut_g=np.asarray(cut_g), T=T,
                NODES_PAD=NODES_PAD, C_LO=C_LO, C_HI=C_HI, C_TOT=C_TOT,
                idx16=idx16, idxd16=idxd16, dloc_f=dloc_f, eaT=eaT, G_T=G_T,
                batch_loc=batch_loc, n_per=n_per)


# ----------------------------------------------------------------------------
# device program builder (shared by both launches)
# ----------------------------------------------------------------------------

def _build_program(layer, S, npos, in_dim, G_T=0, t_lim=None, variant="full"):
    """layer 0: x -> h0_slice.  layer 1: h -> logits (pool + MLP + lsm).

    S: shard-structure dict (C_LO/C_HI/T/...). Weights arrive as inputs.
    """
    T, C_LO, C_HI, C_TOT = S["T"], S["C_LO"], S["C_HI"], S["C_TOT"]
    NODES_PAD = S["NODES_PAD"]
    ID = in_dim

    nc = bacc.Bacc("TRN2", target_bir_lowering=False,
                   dynamic_dma_scratch_size=65536, num_swdge_queues=4)
    dp = nc.declare_dram_parameter
    xT_d = dp("xT", [ID, NG_PAD], F32, isOutput=False)
    xsT_d = dp("xsT", [ID, NODES_PAD], F32, isOutput=False)
    idx_d = dp("idx16", [128, C_TOT * 8], I16, isOutput=False)
    idxd_d = dp("idxd16", [128, C_TOT * 8], I16, isOutput=False)
    dloc_d = dp("dloc", [128, C_TOT], F32, isOutput=False)
    eaT_d = dp("eaT", [EDIM, C_TOT * P], F32, isOutput=False)
    Wl_d = dp("Wl", [ID, H], F32, isOutput=False)
    Wr_d = dp("Wr", [ID, H], F32, isOutput=False)
    We_d = dp("We", [EDIM, H], F32, isOutput=False)
    blr_d = dp("bl_rep", [P, H], F32, isOutput=False)
    brr_d = dp("br_rep", [P, H], F32, isOutput=False)
    bias_d = dp("bias_rep", [P, H], F32, isOutput=False)
    rcp_d = dp("rcp_rep", [P, H], F32, isOutput=False)
    if layer == 0:
        h_d = dp("h_out", [NODES_PAD, H], F32, isOutput=True)
    else:
        bloc_d = dp("batch_loc", [128, T], F32, isOutput=False)
        fc1w_d = dp("fc1w", [H, FC], F32, isOutput=False)
        fc1b_d = dp("fc1b", [FC, 1], F32, isOutput=False)
        fc2w_d = dp("fc2w", [FC, NC_CLS], F32, isOutput=False)
        fc2b_d = dp("fc2b", [NC_CLS, 1], F32, isOutput=False)
        lg_d = dp("logits", [G_T, NC_CLS], F32, isOutput=True)
    xl_lo = nc.dram_tensor("xl_lo", [LO, H], F32)
    xl_hi = nc.dram_tensor("xl_hi", [NHI, H], F32)
    xr_sl = nc.dram_tensor("xr_sl", [NODES_PAD, H], F32)

    with tile.TileContext(nc) as tc, ExitStack() as ctx:
        con = ctx.enter_context(tc.tile_pool(name="con", bufs=1))
        pj = ctx.enter_context(tc.tile_pool(name="pj", bufs=3))
        pjp = ctx.enter_context(tc.tile_pool(name="pjp", bufs=2, space="PSUM"))
        met = ctx.enter_context(tc.tile_pool(name="met", bufs=2))
        gat = ctx.enter_context(tc.tile_pool(name="gat", bufs=2))
        wrk = ctx.enter_context(tc.tile_pool(name="wrk", bufs=4))
        epp = ctx.enter_context(tc.tile_pool(name="epp", bufs=2, space="PSUM"))
        agp = ctx.enter_context(tc.tile_pool(name="agp", bufs=1, space="PSUM"))

        # ---- constants ----
        ident = con.tile([P, P], F32)
        make_identity(nc, ident[:])
        io_i = con.tile([P, P], I32)
        nc.gpsimd.iota(io_i[:], pattern=[[1, P]], base=0, channel_multiplier=0)
        iota_row = con.tile([P, P], F32)
        nc.vector.tensor_copy(iota_row[:], io_i[:])

        Wl_t = con.tile([ID, H], F32); nc.sync.dma_start(Wl_t[:], Wl_d[:])
        Wr_t = con.tile([ID, H], F32); nc.sync.dma_start(Wr_t[:], Wr_d[:])
        We_t = con.tile([EDIM, H], F32); nc.sync.dma_start(We_t[:], We_d[:])
        blr_t = con.tile([P, H], F32); nc.sync.dma_start(blr_t[:], blr_d[:])
        brr_t = con.tile([P, H], F32); nc.sync.dma_start(brr_t[:], brr_d[:])
        bias_t = con.tile([P, H], F32); nc.sync.dma_start(bias_t[:], bias_d[:])
        rcp_t = con.tile([P, H], F32); nc.sync.dma_start(rcp_t[:], rcp_d[:])
        xsT_t = con.tile([ID, NODES_PAD], F32); nc.sync.dma_start(xsT_t[:], xsT_d[:])
        dloc_t = con.tile([128, C_TOT], F32); nc.sync.dma_start(dloc_t[:], dloc_d[:])

        if layer == 1:
            bloc_t = con.tile([128, T], F32); nc.sync.dma_start(bloc_t[:], bloc_d[:])
            giota_i = con.tile([P, G_T], I32)
            nc.gpsimd.iota(giota_i[:], pattern=[[1, G_T]], base=0, channel_multiplier=0)
            giota = con.tile([P, G_T], F32)
            nc.vector.tensor_copy(giota[:], giota_i[:])
            fc1w_t = con.tile([H, FC], F32); nc.sync.dma_start(fc1w_t[:], fc1w_d[:])
            fc1b_t = con.tile([FC, 1], F32); nc.sync.dma_start(fc1b_t[:], fc1b_d[:])
            fc2w_t = con.tile([FC, NC_CLS], F32); nc.sync.dma_start(fc2w_t[:], fc2w_d[:])
            fc2b_t = con.tile([NC_CLS, 1], F32); nc.sync.dma_start(fc2b_t[:], fc2b_d[:])
            plp = ctx.enter_context(tc.tile_pool(name="plp", bufs=1, space="PSUM"))
            pool_ps = plp.tile([G_T, H + 1], F32, space="PSUM", tag="pool")

        # ---- phase 1: xl tables (all global tiles), streamed 4 tiles/load ----
        for g4 in range(0, NT_GLOB, 4):
            ng = min(4, NT_GLOB - g4)
            xs = met.tile([ID, 4 * P], F32, tag="xg")
            nc.sync.dma_start(xs[:, :ng * P], xT_d[:, g4 * P:(g4 + ng) * P])
            for k in range(ng):
                g = g4 + k
                ps = pjp.tile([P, H], F32, space="PSUM", tag="pp")
                nc.tensor.matmul(ps[:], lhsT=xs[:, k * P:(k + 1) * P], rhs=Wl_t[:],
                                 start=True, stop=True)
                sb = pj.tile([P, H], F32, tag="ps")
                nc.vector.tensor_tensor(out=sb[:], in0=ps[:], in1=blr_t[:],
                                        op=mybir.AluOpType.add)
                if g * P < LO:
                    nc.sync.dma_start(xl_lo[g * P:(g + 1) * P, :], sb[:])
                else:
                    o = g * P - LO
                    nc.sync.dma_start(xl_hi[o:o + P, :], sb[:])

        # ---- phase 1b: xr table for this core's node slice ----
        for j in range(T):
            ps = pjp.tile([P, H], F32, space="PSUM", tag="pp")
            nc.tensor.matmul(ps[:], lhsT=xsT_t[:, j * P:(j + 1) * P], rhs=Wr_t[:],
                             start=True, stop=True)
            sb = pj.tile([P, H], F32, tag="ps")
            nc.vector.tensor_tensor(out=sb[:], in0=ps[:], in1=brr_t[:],
                                    op=mybir.AluOpType.add)
            nc.sync.dma_start(xr_sl[j * P:(j + 1) * P, :], sb[:])

        # ---- phase 2: edge tiles ----
        T_RUN = T if t_lim is None else t_lim
        iota_ap = iota_row[:]
        off = 0
        for j in range(T_RUN):
            CL, CH = int(C_LO[j]), int(C_HI[j])
            CJ = CL + CH

            idx_t = met.tile([128, CJ * 8], I16, tag="ix")
            nc.sync.dma_start(idx_t[:], idx_d[:, off * 8:(off + CJ) * 8])
            idxd_t = met.tile([128, CJ * 8], I16, tag="id")
            nc.sync.dma_start(idxd_t[:], idxd_d[:, off * 8:(off + CJ) * 8])

            gxl = gat.tile([P, CJ, H], F32, tag="gx")
            gxr = gat.tile([P, CJ, H], F32, tag="gr")
            if variant == "nogather":
                nc.vector.memset(gxl[:], 0.01)
                nc.vector.memset(gxr[:], 0.01)
            else:
                nc.gpsimd.dma_gather(
                    out_ap=gxl[:, :CL, :], in_ap=xl_lo[:],
                    idxs_ap=idx_t[:, :CL * 8],
                    num_idxs=CL * P, num_idxs_reg=CL * P, elem_size=H,
                    single_packet=False, queue_num=j % 4)
                nc.gpsimd.dma_gather(
                    out_ap=gxl[:, CL:, :], in_ap=xl_hi[:],
                    idxs_ap=idx_t[:, CL * 8:CJ * 8],
                    num_idxs=CH * P, num_idxs_reg=CH * P, elem_size=H,
                    single_packet=False, queue_num=(j + 1) % 4)
                nc.gpsimd.dma_gather(
                    out_ap=gxr[:], in_ap=xr_sl[:],
                    idxs_ap=idxd_t[:, :CJ * 8],
                    num_idxs=CJ * P, num_idxs_reg=CJ * P, elem_size=H,
                    single_packet=False, queue_num=(j + 2) % 4)
            if variant == "gatheronly":
                ht = wrk.tile([P, H], F32, tag="ht")
                nc.vector.tensor_tensor(out=ht[:], in0=gxl[:, 0, :],
                                        in1=gxr[:, CJ - 1, :], op=mybir.AluOpType.add)
                if layer == 0:
                    nc.sync.dma_start(h_d[j * P:(j + 1) * P, :], ht[:])
                off += CJ
                continue

            agg = agp.tile([H + 1, P], F32, space="PSUM", tag="agg")

            for c4 in range(0, CJ, 8):
                nb = min(8, CJ - c4)
                # M_em for nb chunks in one op
                m4 = wrk.tile([P, nb * P], F32, tag="m4")
                iota3 = bass.AP(iota_ap.tensor, iota_ap.offset,
                                [list(iota_ap.ap[0]), [0, nb], list(iota_ap.ap[1])])
                nc.vector.tensor_tensor(
                    out=m4[:].rearrange("p (c q) -> p c q", c=nb),
                    in0=dloc_t[:, off + c4:off + c4 + nb].to_broadcast([P, nb, P]),
                    in1=iota3, op=mybir.AluOpType.is_equal)
                # ea matmuls (per chunk), then e = ep + gxl + gxr (batched)
                eaT_t = met.tile([EDIM, nb * P], F32, tag="ea")
                nc.sync.dma_start(eaT_t[:, :nb * P],
                                  eaT_d[:, (off + c4) * P:(off + c4 + nb) * P])
                ep = epp.tile([P, nb * H], F32, space="PSUM", tag="ep")
                for k in range(nb):
                    nc.tensor.matmul(ep[:, k * H:(k + 1) * H],
                                     lhsT=eaT_t[:, k * P:(k + 1) * P],
                                     rhs=We_t[:], start=True, stop=True)
                e4 = wrk.tile([P, nb * H], F32, tag="e4")
                nc.vector.tensor_tensor(
                    out=e4[:], in0=ep[:],
                    in1=gxl[:, c4:c4 + nb, :].rearrange("p c q -> p (c q)"),
                    op=mybir.AluOpType.add)
                nc.vector.tensor_tensor(
                    out=e4[:], in0=e4[:],
                    in1=gxr[:, c4:c4 + nb, :].rearrange("p c q -> p (c q)"),
                    op=mybir.AluOpType.add)
                # score pieces: relu (one op) + 4 reduces
                scr = wrk.tile([P, nb * H], F32, tag="scr")
                nc.scalar.activation(out=scr[:], in_=e4[:],
                                     func=mybir.ActivationFunctionType.Relu)
                e3 = e4[:].rearrange("p (c q) -> p c q", c=nb)
                s3 = scr[:].rearrange("p (c q) -> p c q", c=nb)
                rp4 = wrk.tile([P, nb], F32, tag="rp")
                rn4 = wrk.tile([P, nb], F32, tag="rn")
                sp4 = wrk.tile([P, nb], F32, tag="sp")
                sn4 = wrk.tile([P, nb], F32, tag="sn")
                nc.vector.tensor_reduce(out=rp4[:], in_=s3[:, :, :npos],
                                        axis=mybir.AxisListType.X, op=mybir.AluOpType.add)
                nc.vector.tensor_reduce(out=rn4[:], in_=s3[:, :, npos:],
                                        axis=mybir.AxisListType.X, op=mybir.AluOpType.add)
                nc.vector.tensor_reduce(out=sp4[:], in_=e3[:, :, :npos],
                                        axis=mybir.AxisListType.X, op=mybir.AluOpType.add)
                nc.vector.tensor_reduce(out=sn4[:], in_=e3[:, :, npos:],
                                        axis=mybir.AxisListType.X, op=mybir.AluOpType.add)
                u4 = wrk.tile([P, nb], F32, tag="u4")
                nc.vector.tensor_scalar(out=u4[:], in0=sp4[:], scalar1=ALPHA,
                                        op0=mybir.AluOpType.mult, scalar2=None)
                nc.vector.tensor_tensor(out=u4[:], in0=u4[:], in1=sn4[:],
                                        op=mybir.AluOpType.add)
                v4 = wrk.tile([P, nb], F32, tag="v4")
                nc.vector.tensor_tensor(out=v4[:], in0=rp4[:], in1=rn4[:],
                                        op=mybir.AluOpType.subtract)
                nc.vector.tensor_scalar(out=v4[:], in0=v4[:], scalar1=1.0 - ALPHA,
                                        op0=mybir.AluOpType.mult, scalar2=None)
                nc.vector.tensor_tensor(out=v4[:], in0=v4[:], in1=u4[:],
                                        op=mybir.AluOpType.add)
                ex4 = wrk.tile([P, nb], F32, tag="ex")
                nc.scalar.activation(out=ex4[:], in_=v4[:],
                                     func=mybir.ActivationFunctionType.Exp)
                # weighted messages [128, nb, 65]
                wm4 = wrk.tile([P, nb * (H + 1)], F32, tag="wm")
                wm4v = wm4[:].rearrange("p (c q) -> p c q", c=nb)
                nc.vector.tensor_tensor(
                    out=wm4v[:, :, :H],
                    in0=gxl[:, c4:c4 + nb, :],
                    in1=ex4[:].to_broadcast([P, nb, H]),
                    op=mybir.AluOpType.mult)
                nc.vector.tensor_copy(out=wm4v[:, :, H:],
                                      in_=ex4[:].to_broadcast([P, nb, 1]))
                for k in range(nb):
                    nc.tensor.matmul(agg[:], lhsT=wm4[:, k * (H + 1):(k + 1) * (H + 1)],
                                     rhs=m4[:, k * P:(k + 1) * P],
                                     start=(c4 + k == 0), stop=(c4 + k == CJ - 1))
            off += CJ

            # ---- tile epilogue ----
            agg_sb = wrk.tile([H + 1, P], F32, tag="agsb")
            nc.vector.tensor_copy(agg_sb[:], agg[:])
            agt_ps = pjp.tile([P, H + 1], F32, space="PSUM", tag="pp")
            nc.tensor.transpose(out=agt_ps[:], in_=agg_sb[:], identity=ident[:H + 1, :H + 1])
            den = wrk.tile([P, 1], F32, tag="den")
            nc.vector.tensor_scalar(out=den[:], in0=agt_ps[:, H:H + 1], scalar1=1e-16,
                                    op0=mybir.AluOpType.add, scalar2=None)
            rden = wrk.tile([P, 1], F32, tag="rden")
            nc.vector.reciprocal(out=rden[:], in_=den[:])
            o1 = wrk.tile([P, H], F32, tag="o1")
            nc.vector.tensor_tensor(out=o1[:], in0=agt_ps[:, :H],
                                    in1=rden[:].to_broadcast([P, H]),
                                    op=mybir.AluOpType.mult)
            nc.vector.tensor_tensor(out=o1[:], in0=o1[:], in1=rcp_t[:],
                                    op=mybir.AluOpType.mult)
            nc.vector.tensor_tensor(out=o1[:], in0=o1[:], in1=bias_t[:],
                                    op=mybir.AluOpType.add)
            # ELU
            vmin = wrk.tile([P, H], F32, tag="vm")
            nc.vector.tensor_scalar(out=vmin[:], in0=o1[:], scalar1=0.0,
                                    op0=mybir.AluOpType.min, scalar2=None)
            ev = wrk.tile([P, H], F32, tag="ev")
            nc.scalar.activation(out=ev[:], in_=vmin[:],
                                 func=mybir.ActivationFunctionType.Exp)
            rl = wrk.tile([P, H], F32, tag="rl")
            nc.scalar.activation(out=rl[:], in_=o1[:],
                                 func=mybir.ActivationFunctionType.Relu)
            ht = wrk.tile([P, H], F32, tag="ht")
            nc.vector.tensor_tensor(out=ht[:], in0=ev[:], in1=rl[:],
                                    op=mybir.AluOpType.add)
            nc.vector.tensor_scalar(out=ht[:], in0=ht[:], scalar1=1.0,
                                    op0=mybir.AluOpType.subtract, scalar2=None)

            if layer == 0:
                nc.sync.dma_start(h_d[j * P:(j + 1) * P, :], ht[:])
            else:
                pg = wrk.tile([P, G_T], F32, tag="pg")
                nc.vector.tensor_tensor(
                    out=pg[:], in0=bloc_t[:, j:j + 1].to_broadcast([P, G_T]),
                    in1=giota[:], op=mybir.AluOpType.is_equal)
                h1e = wrk.tile([P, H + 1], F32, tag="h1e")
                nc.vector.tensor_copy(out=h1e[:, :H], in_=ht[:])
                nc.vector.memset(h1e[:, H:], 1.0)
                nc.tensor.matmul(pool_ps[:], lhsT=pg[:], rhs=h1e[:],
                                 start=(j == 0), stop=(j == T_RUN - 1))

        # ---- launch-B tail: mean, MLP, log_softmax ----
        if layer == 1:
            cnt = wrk.tile([G_T, 1], F32, tag="cnt")
            nc.vector.tensor_scalar(out=cnt[:], in0=pool_ps[:, H:H + 1], scalar1=1.0,
                                    op0=mybir.AluOpType.max, scalar2=None)
            rc = wrk.tile([G_T, 1], F32, tag="rc")
            nc.vector.reciprocal(out=rc[:], in_=cnt[:])
            gm = wrk.tile([G_T, H], F32, tag="gm")
            nc.vector.tensor_tensor(out=gm[:], in0=pool_ps[:, :H],
                                    in1=rc[:].to_broadcast([G_T, H]),
                                    op=mybir.AluOpType.mult)
            gf_ps = pjp.tile([H, G_T], F32, space="PSUM", tag="pp")
            nc.tensor.transpose(out=gf_ps[:], in_=gm[:], identity=ident[:G_T, :G_T])
            gf = wrk.tile([H, G_T], F32, tag="gfs")
            nc.vector.tensor_copy(gf[:], gf_ps[:])
            o1_ps = epp.tile([FC, G_T], F32, space="PSUM", tag="ep")
            nc.tensor.matmul(o1_ps[:], lhsT=fc1w_t[:], rhs=gf[:], start=True, stop=True)
            o1s = wrk.tile([FC, G_T], F32, tag="o1s")
            nc.scalar.activation(out=o1s[:], in_=o1_ps[:],
                                 func=mybir.ActivationFunctionType.Relu,
                                 bias=fc1b_t[:])
            o2_ps = pjp.tile([NC_CLS, G_T], F32, space="PSUM", tag="pp")
            nc.tensor.matmul(o2_ps[:], lhsT=fc2w_t[:], rhs=o1s[:], start=True, stop=True)
            o2s = wrk.tile([NC_CLS, G_T], F32, tag="o2s")
            nc.scalar.activation(out=o2s[:], in_=o2_ps[:],
                                 func=mybir.ActivationFunctionType.Identity,
                                 bias=fc2b_t[:])
            lgt_ps = epp.tile([G_T, NC_CLS], F32, space="PSUM", tag="ep")
            nc.tensor.transpose(out=lgt_ps[:], in_=o2s[:],
                                identity=ident[:NC_CLS, :NC_CLS])
            lg = wrk.tile([G_T, NC_CLS], F32, tag="lg")
            nc.vector.tensor_copy(lg[:], lgt_ps[:])
            mx = wrk.tile([G_T, 1], F32, tag="mx")
            nc.vector.tensor_reduce(out=mx[:], in_=lg[:], axis=mybir.AxisListType.X,
                                    op=mybir.AluOpType.max)
            dd = wrk.tile([G_T, NC_CLS], F32, tag="dd")
            nc.vector.tensor_tensor(out=dd[:], in0=lg[:],
                                    in1=mx[:].to_broadcast([G_T, NC_CLS]),
                                    op=mybir.AluOpType.subtract)
            ee = wrk.tile([G_T, NC_CLS], F32, tag="ee")
            nc.scalar.activation(out=ee[:], in_=dd[:],
                                 func=mybir.ActivationFunctionType.Exp)
            ss = wrk.tile([G_T, 1], F32, tag="ss")
            nc.vector.tensor_reduce(out=ss[:], in_=ee[:], axis=mybir.AxisListType.X,
                                    op=mybir.AluOpType.add)
            ls = wrk.tile([G_T, 1], F32, tag="ls")
            nc.scalar.activation(out=ls[:], in_=ss[:],
                                 func=mybir.ActivationFunctionType.Ln)
            out_t = wrk.tile([G_T, NC_CLS], F32, tag="outt")
            nc.vector.tensor_tensor(out=out_t[:], in0=dd[:],
                                    in1=ls[:].to_broadcast([G_T, NC_CLS]),
                                    op=mybir.AluOpType.subtract)
            nc.sync.dma_start(lg_d[:], out_t[:])

    nc.finalize()
    return nc


# ----------------------------------------------------------------------------
# entry point
# ----------------------------------------------------------------------------

_CACHE = {}


def _run_retry(nc, in_maps, tries=6):
    import time as _time
    last = None
    for t in range(tries):
        try:
            return run_bass_kernel_spmd(nc, in_maps, list(range(NCORES))).results
        except Exception as e:  # transient NRT device wedge; NEFF is cached
            last = e
            _time.sleep(2.0 + t)
    raise last


def kernel(x, edge_index, edge_attr, batch,
           Wl0, bl0, Wr0, br0, We0, att0, bias0,
           Wl1, bl1, Wr1, br1, We1, att1, bias1,
           fc1_w, fc1_b, fc2_w, fc2_b):
    import time as _t
    _tm = {"t0": _t.time()}
    _v = os.environ.get("KTIME") == "1"

    def _mark(name):
        if _v:
            print(f"[ktime] {name}: {_t.time()-_tm['t0']:.2f}s", flush=True)
        _tm["t0"] = _t.time()

    x = np.asarray(x, np.float32)
    S = _prep_shards(np.asarray(edge_index), np.asarray(edge_attr),
                     np.asarray(batch))
    _mark("prep_shards")
    T, NODES_PAD, G_T = S["T"], S["NODES_PAD"], S["G_T"]
    node_cut, cut_g = S["node_cut"], S["cut_g"]

    f0 = _fold_weights(np.asarray(Wl0), np.asarray(bl0), np.asarray(Wr0),
                       np.asarray(br0), np.asarray(We0), np.asarray(att0),
                       np.asarray(bias0), np.arange(IN))
    f1 = _fold_weights(np.asarray(Wl1), np.asarray(bl1), np.asarray(Wr1),
                       np.asarray(br1), np.asarray(We1), np.asarray(att1),
                       np.asarray(bias1), f0["order"])

    key = ("prog", T, S["C_TOT"], G_T, f0["npos"], f1["npos"])
    if key not in _CACHE:
        _CACHE[key] = (_build_program(0, S, f0["npos"], IN),
                       _build_program(1, S, f1["npos"], H, G_T))
    ncA, ncB = _CACHE[key]
    _mark("build")

    xT = np.zeros((IN, NG_PAD), np.float32)
    xT[:, :N] = x.T

    base = dict(Wl=f0["Wl"], Wr=f0["Wr"], We=f0["We"],
                bl_rep=_rep(f0["bl"]), br_rep=_rep(f0["br"]),
                bias_rep=_rep(f0["bias"]), rcp_rep=_rep(f0["rcp"]), xT=xT)
    in_maps = []
    for c in range(NCORES):
        s0 = node_cut[c]
        xsT = np.zeros((IN, NODES_PAD), np.float32)
        nc_ = node_cut[c + 1] - s0
        xsT[:, :nc_] = x[s0:s0 + nc_].T
        m = dict(base)
        m.update(xsT=xsT, idx16=S["idx16"][c], idxd16=S["idxd16"][c],
                 dloc=S["dloc_f"][c], eaT=S["eaT"][c])
        in_maps.append(m)
    _mark("maps_A")
    resA = _run_retry(ncA, in_maps)
    _mark("launch_A")

    h = np.zeros((NG_PAD, H), np.float32)
    for c in range(NCORES):
        s0 = node_cut[c]
        nc_ = node_cut[c + 1] - s0
        h[s0:s0 + nc_] = resA[c]["h_out"][:nc_]
    hT = np.ascontiguousarray(h.T)

    base1 = dict(Wl=f1["Wl"], Wr=f1["Wr"], We=f1["We"],
                 bl_rep=_rep(f1["bl"]), br_rep=_rep(f1["br"]),
                 bias_rep=_rep(f1["bias"]), rcp_rep=_rep(f1["rcp"]), xT=hT,
                 fc1w=np.asarray(fc1_w, np.float32)[f1["order"], :],
                 fc1b=np.asarray(fc1_b, np.float32)[:, None],
                 fc2w=np.asarray(fc2_w, np.float32),
                 fc2b=np.asarray(fc2_b, np.float32)[:, None])
    in_maps = []
    for c in range(NCORES):
        s0 = node_cut[c]
        nc_ = node_cut[c + 1] - s0
        hsT = np.zeros((H, NODES_PAD), np.float32)
        hsT[:, :nc_] = h[s0:s0 + nc_].T
        m = dict(base1)
        m.update(xsT=hsT, idx16=S["idx16"][c], idxd16=S["idxd16"][c],
                 dloc=S["dloc_f"][c], eaT=S["eaT"][c],
                 batch_loc=S["batch_loc"][c])
        in_maps.append(m)
    _mark("maps_B")
    resB = _run_retry(ncB, in_maps)
    _mark("launch_B")

    out = np.zeros((G, NC_CLS), np.float32)
    for c in range(NCORES):
        g0, g1 = int(cut_g[c]), int(cut_g[c + 1])
        out[g0:g1] = resB[c]["logits"][:g1 - g0]
    return out

